# revision 8
# baseline (speedup 1.0000x reference)
"""Causal multi-head attention with relative position bias on 8 Trainium2
NeuronCores.

Problem (full shapes): x[2,2048,1024], rel_bias[16,2048,2048],
w_qkv[1024,3072], b_qkv[3072], w_out[1024,1024], b_out[1024].

Sharding: core = (batch, head-group): 2 batches x 4 head-groups of 4 heads.
Each core computes q/k/v projections for its 4 heads, causal attention with
rel-bias, and a partial output projection through its heads' rows of w_out.
Host sums the 4 partial outputs per batch (the tensor-parallel reduce) and
adds b_out.

Device kernel design notes:
- Scores are computed TRANSPOSED (scoresT[kj,qi] = k.q) so no on-chip
  transposes are needed anywhere: softmax reduction over keys becomes a
  matmul contraction, handled by appending a ones-column to V; the PV matmul
  directly produces the transposed attention output that the out-projection
  needs as its stationary operand.
- exp(score + bias) = exp(score) * exp(bias): host precomputes exp(rel_biasT)
  in bf16 with the causal mask baked in as exact zeros. ACT does a pure exp
  straight from PSUM; DVE multiplies two bf16 SBUF operands at 2x rate.
- Attention runs in 512-query blocks; the two heads of a pair share one
  [128,1024] PSUM score tile so each (qb,kj) step is ONE exp and ONE
  multiply. PV accumulates [65,512] per head (64 v-dims + ones column).
- Normalization is fully off the PV critical path: windows only copy the
  unnormalized attention output (DVE cast) and the denominator row (ACT)
  out of PSUM, so the PE never stalls on softmax bookkeeping and the HAM
  clock gate stays warm. All 1/denom are computed at the end in one batch
  with one reciprocal_approx_fast (single-partition DVE reciprocals are
  ~3.3us each), broadcast across partitions with a K=97 matmul against a
  0/1 selector, and applied during a normalize+out-projection pipeline.
"""

import math
import sys
import types
from contextlib import ExitStack

import ml_dtypes
import numpy as np

B, S, D = 2, 2048, 1024
NH, HD = 16, 64
NCORES = 8
HPC = 4  # heads per core (2 pairs)

_BF16 = ml_dtypes.bfloat16


def _install_ntff_hook():
    """concourse.bass_utils imports antenv.axon_hooks for NTFF tracing under
    axon; this container's antenv lacks that module. Provide it, backed by
    the ctypes hook from trn_agent_boot (if present)."""
    if "antenv.axon_hooks" in sys.modules:
        return
    try:
        import antenv
    except ImportError:
        return
    mod = types.ModuleType("antenv.axon_hooks")
    mod._hook = None
    mod.set_axon_ntff_profile_hook = lambda h: setattr(mod, "_hook", h)
    mod.get_axon_ntff_profile_hook = lambda: mod._hook
    sys.modules["antenv.axon_hooks"] = mod
    antenv.axon_hooks = mod
    try:
        from trn_agent_boot.trn_boot import _ntff_profile_via_ctypes

        h = _ntff_profile_via_ctypes("/opt/axon/libaxon_pjrt.so")
        if h is not None:
            mod._hook = h
    except Exception:
        pass


KC = D // 128   # 8 contraction chunks for the projections
NS4 = S // 512  # 4 s-superblocks
NSC = S // 128  # 16 s-chunks


def _phase_load(ctx, tc, nc, d, has_bqk, has_bv, st):
    """DMA weights + xT into persistent SBUF tiles."""
    from concourse import mybir
    bf = mybir.dt.bfloat16
    f32 = mybir.dt.float32

    xt_pool = ctx.enter_context(tc.tile_pool(name="xt", bufs=KC))
    wqk_pool = ctx.enter_context(tc.tile_pool(name="wqk", bufs=KC))
    wv_pool = ctx.enter_context(tc.tile_pool(name="wv", bufs=KC))
    wo_pool = ctx.enter_context(tc.tile_pool(name="wo", bufs=2))
    const_pool = ctx.enter_context(tc.tile_pool(name="consts", bufs=1))
    den_pool = ctx.enter_context(tc.tile_pool(name="den", bufs=1))

    st.ones_row = const_pool.tile([1, 512], bf)
    nc.gpsimd.memset(st.ones_row[:], 1.0)

    # 0/1 selector for the denominator broadcast matmuls: pair p's slice
    # sel[:, 128p:128p+128] has a 1 at (row 32*(2p+h), cols 64h:64h+64) so
    # a single K=97 matmul against the 1/denom rows broadcasts each head's
    # reciprocals across its 64 attnT partitions (all other rows are zero).
    # Host-prepared.
    st.sel = const_pool.tile([97, 256], bf)
    nc.sync.dma_start(st.sel[:], d.sel[:])

    # denominators: local head hl's row at partition 32*hl. Unused
    # partitions are set to 1.0 so the batched ln/exp stays NaN-free.
    st.denom = den_pool.tile([97, S], f32)
    nc.gpsimd.memset(st.denom[:], 1.0)

    st.wqk_t, st.xt_t, st.wv_t = [], [], []
    for k in range(KC):
        w = wqk_pool.tile([128, 512], bf)
        nc.sync.dma_start(w[:], d.wqk[k * 128:(k + 1) * 128, :])
        st.wqk_t.append(w)
        xt = xt_pool.tile([128, S], bf)
        nc.sync.dma_start(xt[:], d.xT[k * 128:(k + 1) * 128, :])
        st.xt_t.append(xt)
    for k in range(KC):
        # wv is first consumed ~30us in; keep it out of the critical
        # DMA prefix that the first qk accumulation chain waits on
        wv = wv_pool.tile([128, 260], bf)
        nc.sync.dma_start(wv[:], d.wv[k * 128:(k + 1) * 128, :])
        st.wv_t.append(wv)
    st.wo_t = []
    for p in range(2):
        w = wo_pool.tile([128, D], bf)
        nc.sync.dma_start(w[:], d.wo[p])
        st.wo_t.append(w)
    if has_bqk:
        st.bqk_sb = []
        for m in range(4):
            t = const_pool.tile([1, 128], bf, name=f"bqk{m}", tag=f"bqk{m}")
            nc.sync.dma_start(t[:], d.bqk[m:m + 1, :])
            st.bqk_sb.append(t)
    if has_bv:
        st.bv_sb = const_pool.tile([1, 260], bf)
        nc.sync.dma_start(st.bv_sb[:], d.bv[:])


def _phase_proj(ctx, tc, nc, has_bqk, has_bv, st):
    """qkv projections.

    qkT[m][r, s]: m-chunks 0..3 = [q pair0 | k pair0 | q pair1 | k pair1];
    within a chunk rows 0-63 = first head of the pair, 64-127 = second.
    v_t[si]: [128, 260] bf16, 4 slots of 65 cols (64 v-cols + ones col).
    """
    from concourse import mybir
    bf = mybir.dt.bfloat16
    f32 = mybir.dt.float32

    qkT_pool = ctx.enter_context(tc.tile_pool(name="qkT", bufs=4))
    v_pool = ctx.enter_context(tc.tile_pool(name="vsb", bufs=NSC))
    st.qkT_t = [qkT_pool.tile([128, S], bf, name="qkT", tag="qkT") for _ in range(4)]
    st.v_t = [v_pool.tile([128, 260], bf, name="vsb", tag="vsb") for _ in range(NSC)]

    def emit_qk(qk_ps, m):
        for s4 in range(NS4):
            ps = qk_ps.tile([128, 512], f32, name="qkps", tag="qkps")
            for k in range(KC):
                nc.tensor.matmul(
                    ps[:],
                    st.wqk_t[k][:, m * 128:(m + 1) * 128],
                    st.xt_t[k][:, s4 * 512:(s4 + 1) * 512],
                    start=(k == 0),
                    stop=(k == KC - 1 and not has_bqk),
                )
            if has_bqk:
                nc.tensor.matmul(
                    ps[:], st.bqk_sb[m][:], st.ones_row[:, :],
                    start=False, stop=True,
                )
            nc.vector.tensor_copy(
                st.qkT_t[m][:, s4 * 512:(s4 + 1) * 512], ps[:])

    with tc.tile_pool(name="qk_ps", bufs=4, space="PSUM") as qk_ps, \
         tc.tile_pool(name="v_ps", bufs=3, space="PSUM") as v_ps:
        for m in range(4):
            emit_qk(qk_ps, m)
        for si in range(NSC):
            ps = v_ps.tile([128, 260], f32)
            for k in range(KC):
                nc.tensor.matmul(
                    ps[:],
                    st.xt_t[k][:, si * 128:(si + 1) * 128],
                    st.wv_t[k][:],
                    start=(k == 0),
                    stop=(k == KC - 1 and not has_bv),
                )
            if has_bv:
                nc.tensor.matmul(
                    ps[:], st.ones_row[0:1, 0:128], st.bv_sb[:],
                    start=False, stop=True,
                )
            nc.scalar.copy(st.v_t[si][:], ps[:])
            for h in range(HPC):
                nc.gpsimd.memset(st.v_t[si][:, 65 * h + 64:65 * h + 65], 1.0)


def _phase_attn(ctx, tc, nc, d, st):
    """Causal attention in 512-query blocks.

    Per (pair, qb): for each key chunk kj, both heads' transposed scores go
    into one [128,1024] PSUM tile (h0 cols 0-511, h1 cols 512-1023) via
    tile_position-packed K=64 matmuls -> one exp -> one erb multiply -> two
    PV accumulations. At block end the unnormalized [64,512] outputs are
    cast to SBUF and the denominator rows collected; no normalization here.
    """
    from concourse import mybir
    bf = mybir.dt.bfloat16
    f32 = mybir.dt.float32
    EXP = mybir.ActivationFunctionType.Exp

    attnU_pool = ctx.enter_context(tc.tile_pool(name="attnU", bufs=2))
    st.attnU = [attnU_pool.tile([128, S], bf, name="attnU", tag="attnU")
                for _ in range(2)]

    with ExitStack() as cctx:
        sc_ps = cctx.enter_context(tc.tile_pool(name="sc_ps", bufs=2, space="PSUM"))
        pv_ps = cctx.enter_context(tc.tile_pool(name="pv_ps", bufs=4, space="PSUM"))
        erb_pool = cctx.enter_context(tc.tile_pool(name="erb", bufs=8))
        esc_pool = cctx.enter_context(tc.tile_pool(name="esc", bufs=4))
        prob_pool = cctx.enter_context(tc.tile_pool(name="prob", bufs=4))

        for p in range(2):
            qT = st.qkT_t[2 * p]
            kT = st.qkT_t[2 * p + 1]
            for qb in range(4):
                qs = qb * 512
                nkj = 4 * qb + 4
                pv = [pv_ps.tile([65, 512], f32, name="pv", tag="pv")
                      for _ in range(2)]
                for kj in range(nkj):
                    sc = sc_ps.tile([128, 1024], f32, name="sc", tag="sc")
                    for h in range(2):
                        rows = slice(64 * h, 64 * h + 64)
                        nc.tensor.matmul(
                            sc[:, 512 * h:512 * h + 512],
                            kT[rows, kj * 128:(kj + 1) * 128],
                            qT[rows, qs:qs + 512],
                            start=True, stop=True,
                            tile_position=(64 * h, 0),
                        )
                    esc = esc_pool.tile([128, 1024], bf, name="esc", tag="esc")
                    nc.scalar.activation(esc[:], sc[:], EXP)
                    rb = erb_pool.tile([128, 1024], bf, name="erb", tag="erb")
                    for h in range(2):
                        nc.sync.dma_start(
                            rb[:, 512 * h:512 * h + 512],
                            d.erb[2 * p + h, kj * 128:(kj + 1) * 128, qs:qs + 512])
                    pr = prob_pool.tile([128, 1024], bf, name="prob", tag="prob")
                    nc.vector.tensor_mul(pr[:], esc[:], rb[:])
                    for h in range(2):
                        hl = 2 * p + h
                        nc.tensor.matmul(
                            pv[h][:],
                            st.v_t[kj][:, 65 * hl:65 * hl + 65],
                            pr[:, 512 * h:512 * h + 512],
                            start=(kj == 0),
                            stop=(kj == nkj - 1),
                        )
                for h in range(2):
                    nc.vector.tensor_copy(
                        st.attnU[p][64 * h:64 * h + 64, qs:qs + 512],
                        pv[h][0:64, :])
                    hl = 2 * p + h
                    nc.scalar.copy(
                        st.denom[32 * hl:32 * hl + 1, qs:qs + 512],
                        pv[h][64:65, :])


def _phase_norm_out(ctx, tc, nc, d, st):
    """Batched softmax normalization fused with the output projection.

    1/denom for all heads/queries in one shot: rec = exp(-ln(denom)) on ACT
    (both functions live in one table set; single-partition DVE reciprocals
    would cost ~3.3us each). Per 512-query block: a K=2 matmul against the
    0/1 selector broadcasts the two heads' 1/denom rows across the 128
    attnT partitions, one DVE multiply normalizes, then the block's four
    128-query out-projection chunks run.
    """
    from concourse import mybir
    bf = mybir.dt.bfloat16
    f32 = mybir.dt.float32
    EXP = mybir.ActivationFunctionType.Exp
    LN = mybir.ActivationFunctionType.Ln

    rec_pool = ctx.enter_context(tc.tile_pool(name="rec", bufs=1))
    attnT_pool = ctx.enter_context(tc.tile_pool(name="attnT", bufs=2))
    st.attnT = [attnT_pool.tile([128, S], bf, name="attnT", tag="attnT")
                for _ in range(2)]

    lnd = rec_pool.tile([97, S], f32)
    rec = rec_pool.tile([97, S], bf)
    nc.scalar.activation(lnd[:], st.denom[:], LN)
    nc.scalar.activation(rec[:], lnd[:], EXP, scale=-1.0)

    with tc.tile_pool(name="bc_ps", bufs=2, space="PSUM") as bc_ps, \
         tc.tile_pool(name="o_ps", bufs=4, space="PSUM") as o_ps, \
         tc.tile_pool(name="osb", bufs=4) as osb_pool:
        for qb in range(4):
            qs = qb * 512
            for p in range(2):
                bc = bc_ps.tile([128, 512], f32, name="bc", tag="bc")
                nc.tensor.matmul(
                    bc[:],
                    st.sel[:, 128 * p:128 * p + 128],
                    rec[:, qs:qs + 512],
                    start=True, stop=True,
                )
                nc.vector.tensor_mul(
                    st.attnT[p][:, qs:qs + 512],
                    st.attnU[p][:, qs:qs + 512],
                    bc[:])
            for si in range(4 * qb, 4 * qb + 4):
                ps = [o_ps.tile([128, 512], f32, name="ops", tag="ops")
                      for _ in range(2)]
                for pp in range(2):
                    for e2 in range(2):
                        nc.tensor.matmul(
                            ps[e2][:],
                            st.attnT[pp][:, si * 128:(si + 1) * 128],
                            st.wo_t[pp][:, e2 * 512:(e2 + 1) * 512],
                            start=(pp == 0), stop=(pp == 1),
                        )
                for e2 in range(2):
                    osb = osb_pool.tile([128, 512], f32, name="osb", tag="osb")
                    if e2 == 0:
                        nc.vector.tensor_copy(osb[:], ps[e2][:])
                    else:
                        nc.scalar.copy(osb[:], ps[e2][:])
                    nc.sync.dma_start(
                        d.out[si * 128:(si + 1) * 128,
                              e2 * 512:(e2 + 1) * 512],
                        osb[:])


_LDW_OPT_INSTALLED = False


def _enable_ldw_opt():
    """walrus ships with --enable-ldw-opt=false; flip it for this process
    (dedupes/hoists LDWEIGHTS). Gated by KERNEL_LDW_OPT=1."""
    global _LDW_OPT_INSTALLED
    if _LDW_OPT_INSTALLED:
        return
    _LDW_OPT_INSTALLED = True
    import os
    if os.environ.get("KERNEL_LDW_OPT", "0") != "1":
        return
    import concourse.bass_utils as bu
    orig = bu.run_command

    def patched(argv, **kwargs):
        argv = ["--enable-ldw-opt=true" if a == "--enable-ldw-opt=false" else a
                for a in argv]
        return orig(argv, **kwargs)

    bu.run_command = patched


def _build_program(has_bqk: bool, has_bv: bool):
    import concourse.tile as tile
    from concourse import bacc, mybir

    bf = mybir.dt.bfloat16
    f32 = mybir.dt.float32

    nc = bacc.Bacc("TRN2", target_bir_lowering=False, debug=False,
                   num_devices=NCORES)

    d = types.SimpleNamespace()
    d.xT = nc.dram_tensor("xT", [D, S], bf, kind="ExternalInput").ap()
    d.wqk = nc.dram_tensor("wqk", [D, 512], bf, kind="ExternalInput").ap()
    d.wv = nc.dram_tensor("wv", [D, 260], bf, kind="ExternalInput").ap()
    d.bqk = nc.dram_tensor("bqk", [4, 128], bf, kind="ExternalInput").ap()
    d.bv = nc.dram_tensor("bv", [1, 260], bf, kind="ExternalInput").ap()
    d.erb = nc.dram_tensor("erb", [HPC, S, S], bf, kind="ExternalInput").ap()
    d.sel = nc.dram_tensor("sel", [97, 256], bf, kind="ExternalInput").ap()
    d.wo = nc.dram_tensor("wo", [2, 128, D], bf, kind="ExternalInput").ap()
    d.out = nc.dram_tensor("out", [S, D], f32, kind="ExternalOutput").ap()

    st = types.SimpleNamespace()
    with tile.TileContext(nc) as tc:
        with ExitStack() as ctx:
            _phase_load(ctx, tc, nc, d, has_bqk, has_bv, st)
            _phase_proj(ctx, tc, nc, has_bqk, has_bv, st)
            _phase_attn(ctx, tc, nc, d, st)
            _phase_norm_out(ctx, tc, nc, d, st)

    nc.compile()
    return nc


_PROGRAM_CACHE = {}


def _get_program(has_bqk, has_bv):
    key = (has_bqk, has_bv)
    if key not in _PROGRAM_CACHE:
        _PROGRAM_CACHE[key] = _build_program(has_bqk, has_bv)
    return _PROGRAM_CACHE[key]


_last_results = None  # BassKernelResults of the most recent run (for test.py)


def kernel(x, rel_bias, w_qkv, b_qkv, w_out, b_out, *, trace=False):
    global _last_results
    _install_ntff_hook()
    _enable_ldw_opt()
    from concourse.bass_utils import run_bass_kernel_spmd

    x = np.asarray(x, dtype=np.float32)
    rel_bias = np.asarray(rel_bias, dtype=np.float32)
    w_qkv = np.asarray(w_qkv, dtype=np.float32)
    b_qkv = np.asarray(b_qkv, dtype=np.float32)
    w_out = np.asarray(w_out, dtype=np.float32)
    b_out = np.asarray(b_out, dtype=np.float32)

    wq = w_qkv[:, 0:D]
    wk = w_qkv[:, D:2 * D]
    wv = w_qkv[:, 2 * D:3 * D]
    bq, bk, bv = b_qkv[0:D], b_qkv[D:2 * D], b_qkv[2 * D:3 * D]
    has_bqk = bool(np.any(bq)) or bool(np.any(bk))
    has_bv = bool(np.any(bv))

    nc = _get_program(has_bqk, has_bv)

    sc = 1.0 / math.sqrt(HD)  # folded into the q projection
    xT = [np.ascontiguousarray(x[b].T).astype(_BF16) for b in range(B)]
    tri = np.triu(np.ones((S, S), dtype=np.float32))  # [kj, qi]: qi >= kj

    sel_np = np.zeros((97, 256), dtype=_BF16)
    for hl in range(4):
        p, h = divmod(hl, 2)
        sel_np[32 * hl, 128 * p + 64 * h:128 * p + 64 * h + 64] = 1.0

    in_maps = []
    for c in range(NCORES):
        b, hg = divmod(c, 4)
        hs = [4 * hg + i for i in range(HPC)]

        # wqk columns: [q_h0 | q_h1 | k_h0 | k_h1 | q_h2 | q_h3 | k_h2 | k_h3]
        cols = []
        bqk_rows = []
        for pair in range(2):
            h0, h1 = hs[2 * pair], hs[2 * pair + 1]
            cols += [wq[:, HD * h0:HD * (h0 + 1)] * sc,
                     wq[:, HD * h1:HD * (h1 + 1)] * sc]
            bqk_rows.append(np.concatenate(
                [bq[HD * h0:HD * (h0 + 1)], bq[HD * h1:HD * (h1 + 1)]]) * sc)
            cols += [wk[:, HD * h0:HD * (h0 + 1)],
                     wk[:, HD * h1:HD * (h1 + 1)]]
            bqk_rows.append(np.concatenate(
                [bk[HD * h0:HD * (h0 + 1)], bk[HD * h1:HD * (h1 + 1)]]))
        wqk_c = np.concatenate(cols, axis=1).astype(_BF16)
        bqk_c = np.stack(bqk_rows).astype(_BF16)

        wv_c = np.zeros((D, 260), dtype=np.float32)
        bv_c = np.zeros((1, 260), dtype=np.float32)
        for i, h in enumerate(hs):
            wv_c[:, 65 * i:65 * i + 64] = wv[:, HD * h:HD * (h + 1)]
            bv_c[0, 65 * i:65 * i + 64] = bv[HD * h:HD * (h + 1)]

        erb_c = np.empty((HPC, S, S), dtype=_BF16)
        for i, h in enumerate(hs):
            erb_c[i] = (np.exp(rel_bias[h].T) * tri).astype(_BF16)

        in_maps.append({
            "xT": xT[b],
            "wqk": wqk_c,
            "wv": wv_c.astype(_BF16),
            "bqk": bqk_c,
            "bv": bv_c.astype(_BF16),
            "erb": erb_c,
            "sel": sel_np,
            "wo": np.ascontiguousarray(
                w_out[256 * hg:256 * (hg + 1)].reshape(2, 128, D)).astype(_BF16),
        })

    res = run_bass_kernel_spmd(nc, in_maps, list(range(NCORES)), trace=trace)
    _last_results = res

    out = np.zeros((B, S, D), dtype=np.float32)
    for c in range(NCORES):
        out[c // 4] += res.results[c]["out"]
    out += b_out
    return out


# revision 9
# speedup vs baseline: 1.0470x; 1.0470x over previous
"""Causal multi-head attention with relative position bias on 8 Trainium2
NeuronCores.

Problem (full shapes): x[2,2048,1024], rel_bias[16,2048,2048],
w_qkv[1024,3072], b_qkv[3072], w_out[1024,1024], b_out[1024].

Sharding: core = (batch, head-group): 2 batches x 4 head-groups of 4 heads.
Each core computes q/k/v projections for its 4 heads, causal attention with
rel-bias, and a partial output projection through its heads' rows of w_out.
Host sums the 4 partial outputs per batch (the tensor-parallel reduce) and
adds b_out.

Device kernel design notes:
- Scores are computed TRANSPOSED (scoresT[kj,qi] = k.q) so no on-chip
  transposes are needed anywhere: softmax reduction over keys becomes a
  matmul contraction, handled by appending a ones-column to V; the PV matmul
  directly produces the transposed attention output that the out-projection
  needs as its stationary operand.
- exp(score + bias) = exp(score) * exp(bias): host precomputes exp(rel_biasT)
  in bf16 with the causal mask baked in as exact zeros. ACT does a pure exp
  straight from PSUM; DVE multiplies two bf16 SBUF operands at 2x rate.
- Attention runs in 512-query blocks; the two heads of a pair share one
  [128,1024] PSUM score tile so each (qb,kj) step is ONE exp and ONE
  multiply. PV accumulates [65,512] per head (64 v-dims + ones column).
- Normalization is fully off the PV critical path: windows only copy the
  unnormalized attention output (DVE cast) and the denominator row (ACT)
  out of PSUM, so the PE never stalls on softmax bookkeeping and the HAM
  clock gate stays warm. All 1/denom are computed at the end in one batch
  with one reciprocal_approx_fast (single-partition DVE reciprocals are
  ~3.3us each), broadcast across partitions with a K=97 matmul against a
  0/1 selector, and applied during a normalize+out-projection pipeline.
"""

import math
import sys
import types
from contextlib import ExitStack

import ml_dtypes
import numpy as np

B, S, D = 2, 2048, 1024
NH, HD = 16, 64
NCORES = 8
HPC = 4  # heads per core (2 pairs)

_BF16 = ml_dtypes.bfloat16


def _install_ntff_hook():
    """concourse.bass_utils imports antenv.axon_hooks for NTFF tracing under
    axon; this container's antenv lacks that module. Provide it, backed by
    the ctypes hook from trn_agent_boot (if present)."""
    if "antenv.axon_hooks" in sys.modules:
        return
    try:
        import antenv
    except ImportError:
        return
    mod = types.ModuleType("antenv.axon_hooks")
    mod._hook = None
    mod.set_axon_ntff_profile_hook = lambda h: setattr(mod, "_hook", h)
    mod.get_axon_ntff_profile_hook = lambda: mod._hook
    sys.modules["antenv.axon_hooks"] = mod
    antenv.axon_hooks = mod
    try:
        from trn_agent_boot.trn_boot import _ntff_profile_via_ctypes

        h = _ntff_profile_via_ctypes("/opt/axon/libaxon_pjrt.so")
        if h is not None:
            mod._hook = h
    except Exception:
        pass


KC = D // 128   # 8 contraction chunks for the projections
NS4 = S // 512  # 4 s-superblocks
NSC = S // 128  # 16 s-chunks


def _phase_load(ctx, tc, nc, d, has_bqk, has_bv, st):
    """DMA weights + xT into persistent SBUF tiles."""
    from concourse import mybir
    bf = mybir.dt.bfloat16
    f32 = mybir.dt.float32

    xt_pool = ctx.enter_context(tc.tile_pool(name="xt", bufs=KC))
    wqk_pool = ctx.enter_context(tc.tile_pool(name="wqk", bufs=KC))
    wv_pool = ctx.enter_context(tc.tile_pool(name="wv", bufs=KC))
    wo_pool = ctx.enter_context(tc.tile_pool(name="wo", bufs=2))
    const_pool = ctx.enter_context(tc.tile_pool(name="consts", bufs=1))
    den_pool = ctx.enter_context(tc.tile_pool(name="den", bufs=1))

    st.ones_row = const_pool.tile([1, 512], bf)
    nc.gpsimd.memset(st.ones_row[:], 1.0)

    # 0/1 selector for the denominator broadcast matmuls: pair p's slice
    # sel[:, 128p:128p+128] has a 1 at (row 32*(2p+h), cols 64h:64h+64) so
    # a single K=97 matmul against the 1/denom rows broadcasts each head's
    # reciprocals across its 64 attnT partitions (all other rows are zero).
    # Host-prepared.
    st.sel = const_pool.tile([97, 256], bf)
    nc.sync.dma_start(st.sel[:], d.sel[:])

    # denominators: local head hl's row at partition 32*hl. Unused
    # partitions are set to 1.0 so the batched ln/exp stays NaN-free.
    st.denom = den_pool.tile([97, S], f32)
    nc.gpsimd.memset(st.denom[:], 1.0)

    st.wqk_t, st.xt_t, st.wv_t = [], [], []
    for k in range(KC):
        w = wqk_pool.tile([128, 512], bf)
        nc.sync.dma_start(w[:], d.wqk[k * 128:(k + 1) * 128, :])
        st.wqk_t.append(w)
        xt = xt_pool.tile([128, S], bf)
        nc.sync.dma_start(xt[:], d.xT[k * 128:(k + 1) * 128, :])
        st.xt_t.append(xt)
    for k in range(KC):
        # wv is first consumed ~30us in; keep it out of the critical
        # DMA prefix that the first qk accumulation chain waits on
        wv = wv_pool.tile([128, 260], bf)
        nc.sync.dma_start(wv[:], d.wv[k * 128:(k + 1) * 128, :])
        st.wv_t.append(wv)
    st.wo_t = []
    for p in range(2):
        w = wo_pool.tile([128, D], bf)
        nc.sync.dma_start(w[:], d.wo[p])
        st.wo_t.append(w)
    if has_bqk:
        st.bqk_sb = []
        for m in range(4):
            t = const_pool.tile([1, 128], bf, name=f"bqk{m}", tag=f"bqk{m}")
            nc.sync.dma_start(t[:], d.bqk[m:m + 1, :])
            st.bqk_sb.append(t)
    if has_bv:
        st.bv_sb = const_pool.tile([1, 260], bf)
        nc.sync.dma_start(st.bv_sb[:], d.bv[:])


def _phase_proj(ctx, tc, nc, has_bqk, has_bv, st):
    """qkv projections.

    qkT[m][r, s]: m-chunks 0..3 = [q pair0 | k pair0 | q pair1 | k pair1];
    within a chunk rows 0-63 = first head of the pair, 64-127 = second.
    v_t[si]: [128, 260] bf16, 4 slots of 65 cols (64 v-cols + ones col).
    """
    from concourse import mybir
    bf = mybir.dt.bfloat16
    f32 = mybir.dt.float32

    qkT_pool = ctx.enter_context(tc.tile_pool(name="qkT", bufs=4))
    v_pool = ctx.enter_context(tc.tile_pool(name="vsb", bufs=NSC))
    st.qkT_t = [qkT_pool.tile([128, S], bf, name="qkT", tag="qkT") for _ in range(4)]
    st.v_t = [v_pool.tile([128, 260], bf, name="vsb", tag="vsb") for _ in range(NSC)]

    def emit_qk(qk_ps, m):
        for s4 in range(NS4):
            ps = qk_ps.tile([128, 512], f32, name="qkps", tag="qkps")
            for k in range(KC):
                nc.tensor.matmul(
                    ps[:],
                    st.wqk_t[k][:, m * 128:(m + 1) * 128],
                    st.xt_t[k][:, s4 * 512:(s4 + 1) * 512],
                    start=(k == 0),
                    stop=(k == KC - 1 and not has_bqk),
                )
            if has_bqk:
                nc.tensor.matmul(
                    ps[:], st.bqk_sb[m][:], st.ones_row[:, :],
                    start=False, stop=True,
                )
            nc.scalar.copy(
                st.qkT_t[m][:, s4 * 512:(s4 + 1) * 512], ps[:])

    with tc.tile_pool(name="qk_ps", bufs=4, space="PSUM") as qk_ps, \
         tc.tile_pool(name="v_ps", bufs=3, space="PSUM") as v_ps:
        for m in range(4):
            emit_qk(qk_ps, m)
        for si in range(NSC):
            ps = v_ps.tile([128, 260], f32)
            for k in range(KC):
                nc.tensor.matmul(
                    ps[:],
                    st.xt_t[k][:, si * 128:(si + 1) * 128],
                    st.wv_t[k][:],
                    start=(k == 0),
                    stop=(k == KC - 1 and not has_bv),
                )
            if has_bv:
                nc.tensor.matmul(
                    ps[:], st.ones_row[0:1, 0:128], st.bv_sb[:],
                    start=False, stop=True,
                )
            nc.vector.tensor_copy(st.v_t[si][:], ps[:])
            for h in range(HPC):
                nc.gpsimd.memset(st.v_t[si][:, 65 * h + 64:65 * h + 65], 1.0)


def _phase_attn(ctx, tc, nc, d, st):
    """Causal attention in 512-query blocks.

    Per (pair, qb): for each key chunk kj, both heads' transposed scores go
    into one [128,1024] PSUM tile (h0 cols 0-511, h1 cols 512-1023) via
    tile_position-packed K=64 matmuls -> one exp -> one erb multiply -> two
    PV accumulations. At block end the unnormalized [64,512] outputs are
    cast to SBUF and the denominator rows collected; no normalization here.
    """
    from concourse import mybir
    bf = mybir.dt.bfloat16
    f32 = mybir.dt.float32
    EXP = mybir.ActivationFunctionType.Exp

    attnU_pool = ctx.enter_context(tc.tile_pool(name="attnU", bufs=2))
    st.attnU = [attnU_pool.tile([128, S], bf, name="attnU", tag="attnU")
                for _ in range(2)]

    with ExitStack() as cctx:
        sc_ps = cctx.enter_context(tc.tile_pool(name="sc_ps", bufs=2, space="PSUM"))
        pv_ps = cctx.enter_context(tc.tile_pool(name="pv_ps", bufs=4, space="PSUM"))
        erb_pool = cctx.enter_context(tc.tile_pool(name="erb", bufs=8))
        esc_pool = cctx.enter_context(tc.tile_pool(name="esc", bufs=4))
        prob_pool = cctx.enter_context(tc.tile_pool(name="prob", bufs=4))

        for p in range(2):
            qT = st.qkT_t[2 * p]
            kT = st.qkT_t[2 * p + 1]
            for qb in range(4):
                qs = qb * 512
                nkj = 4 * qb + 4
                pv = [pv_ps.tile([65, 512], f32, name="pv", tag="pv")
                      for _ in range(2)]
                for kj in range(nkj):
                    # diagonal key chunks only cover queries >= 128*kj:
                    # clip everything to the causal region at 128-column
                    # granularity (kj==0 has off==0 -> full width).
                    off = max(0, (kj - 4 * qb) * 128)
                    w = 512 - off
                    sc = sc_ps.tile([128, 1024], f32, name="sc", tag="sc")
                    for h in range(2):
                        rows = slice(64 * h, 64 * h + 64)
                        nc.tensor.matmul(
                            sc[:, 512 * h + off:512 * h + 512],
                            kT[rows, kj * 128:(kj + 1) * 128],
                            qT[rows, qs + off:qs + 512],
                            start=True, stop=True,
                            tile_position=(64 * h, 0),
                        )
                    esc = esc_pool.tile([128, 1024], bf, name="esc", tag="esc")
                    if off == 0:
                        nc.scalar.activation(esc[:], sc[:], EXP)
                    else:
                        for h in range(2):
                            nc.scalar.activation(
                                esc[:, 512 * h + off:512 * h + 512],
                                sc[:, 512 * h + off:512 * h + 512], EXP)
                    rb = erb_pool.tile([128, 1024], bf, name="erb", tag="erb")
                    for h in range(2):
                        nc.sync.dma_start(
                            rb[:, 512 * h + off:512 * h + 512],
                            d.erb[2 * p + h, kj * 128:(kj + 1) * 128,
                                  qs + off:qs + 512])
                    pr = prob_pool.tile([128, 1024], bf, name="prob", tag="prob")
                    if off == 0:
                        nc.vector.tensor_mul(pr[:], esc[:], rb[:])
                    else:
                        for h in range(2):
                            nc.vector.tensor_mul(
                                pr[:, 512 * h + off:512 * h + 512],
                                esc[:, 512 * h + off:512 * h + 512],
                                rb[:, 512 * h + off:512 * h + 512])
                    for h in range(2):
                        hl = 2 * p + h
                        nc.tensor.matmul(
                            pv[h][:, off:512],
                            st.v_t[kj][:, 65 * hl:65 * hl + 65],
                            pr[:, 512 * h + off:512 * h + 512],
                            start=(kj == 0),
                            stop=(kj == nkj - 1),
                        )
                for h in range(2):
                    nc.vector.tensor_copy(
                        st.attnU[p][64 * h:64 * h + 64, qs:qs + 512],
                        pv[h][0:64, :])
                    hl = 2 * p + h
                    nc.scalar.copy(
                        st.denom[32 * hl:32 * hl + 1, qs:qs + 512],
                        pv[h][64:65, :])


def _phase_norm_out(ctx, tc, nc, d, st):
    """Batched softmax normalization fused with the output projection.

    1/denom for all heads/queries in one shot: rec = exp(-ln(denom)) on ACT
    (both functions live in one table set; single-partition DVE reciprocals
    would cost ~3.3us each). Per 512-query block: a K=2 matmul against the
    0/1 selector broadcasts the two heads' 1/denom rows across the 128
    attnT partitions, one DVE multiply normalizes, then the block's four
    128-query out-projection chunks run.
    """
    from concourse import mybir
    bf = mybir.dt.bfloat16
    f32 = mybir.dt.float32

    rec_pool = ctx.enter_context(tc.tile_pool(name="rec", bufs=1))
    attnT_pool = ctx.enter_context(tc.tile_pool(name="attnT", bufs=2))
    st.attnT = [attnT_pool.tile([128, S], bf, name="attnT", tag="attnT")
                for _ in range(2)]

    recf = rec_pool.tile([97, S], f32)
    rec = rec_pool.tile([97, S], bf)
    nc.vector.reciprocal_approx_fast(recf[:], st.denom[:])
    nc.vector.tensor_copy(rec[:], recf[:])

    with tc.tile_pool(name="bc_ps", bufs=2, space="PSUM") as bc_ps, \
         tc.tile_pool(name="o_ps", bufs=4, space="PSUM") as o_ps, \
         tc.tile_pool(name="osb", bufs=4) as osb_pool:
        for qb in range(4):
            qs = qb * 512
            for p in range(2):
                bc = bc_ps.tile([128, 512], f32, name="bc", tag="bc")
                nc.tensor.matmul(
                    bc[:],
                    st.sel[:, 128 * p:128 * p + 128],
                    rec[:, qs:qs + 512],
                    start=True, stop=True,
                )
                nc.vector.tensor_mul(
                    st.attnT[p][:, qs:qs + 512],
                    st.attnU[p][:, qs:qs + 512],
                    bc[:])
            for si in range(4 * qb, 4 * qb + 4):
                ps = [o_ps.tile([128, 512], f32, name="ops", tag="ops")
                      for _ in range(2)]
                for pp in range(2):
                    for e2 in range(2):
                        nc.tensor.matmul(
                            ps[e2][:],
                            st.attnT[pp][:, si * 128:(si + 1) * 128],
                            st.wo_t[pp][:, e2 * 512:(e2 + 1) * 512],
                            start=(pp == 0), stop=(pp == 1),
                        )
                for e2 in range(2):
                    osb = osb_pool.tile([128, 512], bf, name="osb", tag="osb")
                    if e2 == 0:
                        nc.vector.tensor_copy(osb[:], ps[e2][:])
                    else:
                        nc.scalar.copy(osb[:], ps[e2][:])
                    nc.sync.dma_start(
                        d.out[si * 128:(si + 1) * 128,
                              e2 * 512:(e2 + 1) * 512],
                        osb[:])


_LDW_OPT_INSTALLED = False


def _enable_ldw_opt():
    """walrus ships with --enable-ldw-opt=false; flip it for this process
    (dedupes/hoists LDWEIGHTS). Gated by KERNEL_LDW_OPT=1."""
    global _LDW_OPT_INSTALLED
    if _LDW_OPT_INSTALLED:
        return
    _LDW_OPT_INSTALLED = True
    import os
    if os.environ.get("KERNEL_LDW_OPT", "0") != "1":
        return
    import concourse.bass_utils as bu
    orig = bu.run_command

    def patched(argv, **kwargs):
        argv = ["--enable-ldw-opt=true" if a == "--enable-ldw-opt=false" else a
                for a in argv]
        return orig(argv, **kwargs)

    bu.run_command = patched


def _build_program(has_bqk: bool, has_bv: bool):
    import concourse.tile as tile
    from concourse import bacc, mybir

    bf = mybir.dt.bfloat16
    f32 = mybir.dt.float32

    nc = bacc.Bacc("TRN2", target_bir_lowering=False, debug=False,
                   num_devices=NCORES)

    d = types.SimpleNamespace()
    d.xT = nc.dram_tensor("xT", [D, S], bf, kind="ExternalInput").ap()
    d.wqk = nc.dram_tensor("wqk", [D, 512], bf, kind="ExternalInput").ap()
    d.wv = nc.dram_tensor("wv", [D, 260], bf, kind="ExternalInput").ap()
    d.bqk = nc.dram_tensor("bqk", [4, 128], bf, kind="ExternalInput").ap()
    d.bv = nc.dram_tensor("bv", [1, 260], bf, kind="ExternalInput").ap()
    d.erb = nc.dram_tensor("erb", [HPC, S, S], bf, kind="ExternalInput").ap()
    d.sel = nc.dram_tensor("sel", [97, 256], bf, kind="ExternalInput").ap()
    d.wo = nc.dram_tensor("wo", [2, 128, D], bf, kind="ExternalInput").ap()
    d.out = nc.dram_tensor("out", [S, D], bf, kind="ExternalOutput").ap()

    st = types.SimpleNamespace()
    with tile.TileContext(nc) as tc:
        with ExitStack() as ctx:
            _phase_load(ctx, tc, nc, d, has_bqk, has_bv, st)
            _phase_proj(ctx, tc, nc, has_bqk, has_bv, st)
            _phase_attn(ctx, tc, nc, d, st)
            _phase_norm_out(ctx, tc, nc, d, st)

    nc.compile()
    return nc


_PROGRAM_CACHE = {}


def _get_program(has_bqk, has_bv):
    key = (has_bqk, has_bv)
    if key not in _PROGRAM_CACHE:
        _PROGRAM_CACHE[key] = _build_program(has_bqk, has_bv)
    return _PROGRAM_CACHE[key]


_last_results = None  # BassKernelResults of the most recent run (for test.py)


def kernel(x, rel_bias, w_qkv, b_qkv, w_out, b_out, *, trace=False):
    global _last_results
    _install_ntff_hook()
    _enable_ldw_opt()
    from concourse.bass_utils import run_bass_kernel_spmd

    x = np.asarray(x, dtype=np.float32)
    rel_bias = np.asarray(rel_bias, dtype=np.float32)
    w_qkv = np.asarray(w_qkv, dtype=np.float32)
    b_qkv = np.asarray(b_qkv, dtype=np.float32)
    w_out = np.asarray(w_out, dtype=np.float32)
    b_out = np.asarray(b_out, dtype=np.float32)

    wq = w_qkv[:, 0:D]
    wk = w_qkv[:, D:2 * D]
    wv = w_qkv[:, 2 * D:3 * D]
    bq, bk, bv = b_qkv[0:D], b_qkv[D:2 * D], b_qkv[2 * D:3 * D]
    has_bqk = bool(np.any(bq)) or bool(np.any(bk))
    has_bv = bool(np.any(bv))

    nc = _get_program(has_bqk, has_bv)

    sc = 1.0 / math.sqrt(HD)  # folded into the q projection
    xT = [np.ascontiguousarray(x[b].T).astype(_BF16) for b in range(B)]
    tri = np.triu(np.ones((S, S), dtype=np.float32))  # [kj, qi]: qi >= kj

    sel_np = np.zeros((97, 256), dtype=_BF16)
    for hl in range(4):
        p, h = divmod(hl, 2)
        sel_np[32 * hl, 128 * p + 64 * h:128 * p + 64 * h + 64] = 1.0

    in_maps = []
    for c in range(NCORES):
        b, hg = divmod(c, 4)
        hs = [4 * hg + i for i in range(HPC)]

        # wqk columns: [q_h0 | q_h1 | k_h0 | k_h1 | q_h2 | q_h3 | k_h2 | k_h3]
        cols = []
        bqk_rows = []
        for pair in range(2):
            h0, h1 = hs[2 * pair], hs[2 * pair + 1]
            cols += [wq[:, HD * h0:HD * (h0 + 1)] * sc,
                     wq[:, HD * h1:HD * (h1 + 1)] * sc]
            bqk_rows.append(np.concatenate(
                [bq[HD * h0:HD * (h0 + 1)], bq[HD * h1:HD * (h1 + 1)]]) * sc)
            cols += [wk[:, HD * h0:HD * (h0 + 1)],
                     wk[:, HD * h1:HD * (h1 + 1)]]
            bqk_rows.append(np.concatenate(
                [bk[HD * h0:HD * (h0 + 1)], bk[HD * h1:HD * (h1 + 1)]]))
        wqk_c = np.concatenate(cols, axis=1).astype(_BF16)
        bqk_c = np.stack(bqk_rows).astype(_BF16)

        wv_c = np.zeros((D, 260), dtype=np.float32)
        bv_c = np.zeros((1, 260), dtype=np.float32)
        for i, h in enumerate(hs):
            wv_c[:, 65 * i:65 * i + 64] = wv[:, HD * h:HD * (h + 1)]
            bv_c[0, 65 * i:65 * i + 64] = bv[HD * h:HD * (h + 1)]

        erb_c = np.empty((HPC, S, S), dtype=_BF16)
        for i, h in enumerate(hs):
            erb_c[i] = (np.exp(rel_bias[h].T) * tri).astype(_BF16)

        in_maps.append({
            "xT": xT[b],
            "wqk": wqk_c,
            "wv": wv_c.astype(_BF16),
            "bqk": bqk_c,
            "bv": bv_c.astype(_BF16),
            "erb": erb_c,
            "sel": sel_np,
            "wo": np.ascontiguousarray(
                w_out[256 * hg:256 * (hg + 1)].reshape(2, 128, D)).astype(_BF16),
        })

    res = run_bass_kernel_spmd(nc, in_maps, list(range(NCORES)), trace=trace)
    _last_results = res

    out = np.zeros((B, S, D), dtype=np.float32)
    for c in range(NCORES):
        out[c // 4] += res.results[c]["out"].astype(np.float32)
    out += b_out
    return out


# revision 11
# speedup vs baseline: 1.0547x; 1.0074x over previous
"""Causal multi-head attention with relative position bias on 8 Trainium2
NeuronCores.

Problem (full shapes): x[2,2048,1024], rel_bias[16,2048,2048],
w_qkv[1024,3072], b_qkv[3072], w_out[1024,1024], b_out[1024].

Sharding: core = (batch, head-group): 2 batches x 4 head-groups of 4 heads.
Each core computes q/k/v projections for its 4 heads, causal attention with
rel-bias, and a partial output projection through its heads' rows of w_out.
Host sums the 4 partial outputs per batch (the tensor-parallel reduce) and
adds b_out.

Device kernel design notes:
- Scores are computed TRANSPOSED (scoresT[kj,qi] = k.q) so no on-chip
  transposes are needed anywhere: softmax reduction over keys becomes a
  matmul contraction, handled by appending a ones-column to V; the PV matmul
  directly produces the transposed attention output that the out-projection
  needs as its stationary operand.
- exp(score + bias) = exp(score) * exp(bias): host precomputes exp(rel_biasT)
  in bf16 with the causal mask baked in as exact zeros. ACT does a pure exp
  straight from PSUM; DVE multiplies two bf16 SBUF operands at 2x rate.
- Attention runs in 512-query blocks; the two heads of a pair share one
  [128,1024] PSUM score tile so each (qb,kj) step is ONE exp and ONE
  multiply. PV accumulates [65,512] per head (64 v-dims + ones column).
- Normalization is fully off the PV critical path: windows only copy the
  unnormalized attention output (DVE cast) and the denominator row (ACT)
  out of PSUM, so the PE never stalls on softmax bookkeeping and the HAM
  clock gate stays warm. All 1/denom are computed at the end in one batch
  with one reciprocal_approx_fast (single-partition DVE reciprocals are
  ~3.3us each), broadcast across partitions with a K=97 matmul against a
  0/1 selector, and applied during a normalize+out-projection pipeline.
"""

import math
import sys
import types
from contextlib import ExitStack

import ml_dtypes
import numpy as np

B, S, D = 2, 2048, 1024
NH, HD = 16, 64
NCORES = 8
HPC = 4  # heads per core (2 pairs)

_BF16 = ml_dtypes.bfloat16


def _install_ntff_hook():
    """concourse.bass_utils imports antenv.axon_hooks for NTFF tracing under
    axon; this container's antenv lacks that module. Provide it, backed by
    the ctypes hook from trn_agent_boot (if present)."""
    if "antenv.axon_hooks" in sys.modules:
        return
    try:
        import antenv
    except ImportError:
        return
    mod = types.ModuleType("antenv.axon_hooks")
    mod._hook = None
    mod.set_axon_ntff_profile_hook = lambda h: setattr(mod, "_hook", h)
    mod.get_axon_ntff_profile_hook = lambda: mod._hook
    sys.modules["antenv.axon_hooks"] = mod
    antenv.axon_hooks = mod
    try:
        from trn_agent_boot.trn_boot import _ntff_profile_via_ctypes

        h = _ntff_profile_via_ctypes("/opt/axon/libaxon_pjrt.so")
        if h is not None:
            mod._hook = h
    except Exception:
        pass


KC = D // 128   # 8 contraction chunks for the projections
NS4 = S // 512  # 4 s-superblocks
NSC = S // 128  # 16 s-chunks


def _phase_load(ctx, tc, nc, d, has_bqk, has_bv, st):
    """DMA weights + xT into persistent SBUF tiles."""
    from concourse import mybir
    bf = mybir.dt.bfloat16
    f32 = mybir.dt.float32

    xt_pool = ctx.enter_context(tc.tile_pool(name="xt", bufs=KC))
    wqk_pool = ctx.enter_context(tc.tile_pool(name="wqk", bufs=KC))
    wv_pool = ctx.enter_context(tc.tile_pool(name="wv", bufs=KC))
    wo_pool = ctx.enter_context(tc.tile_pool(name="wo", bufs=2))
    const_pool = ctx.enter_context(tc.tile_pool(name="consts", bufs=1))
    den_pool = ctx.enter_context(tc.tile_pool(name="den", bufs=1))

    st.ones_row = const_pool.tile([1, 512], bf)
    nc.gpsimd.memset(st.ones_row[:], 1.0)

    # 0/1 selector for the denominator broadcast matmuls: pair p's slice
    # sel[:, 128p:128p+128] has a 1 at (row 32*(2p+h), cols 64h:64h+64) so
    # a single K=97 matmul against the 1/denom rows broadcasts each head's
    # reciprocals across its 64 attnT partitions (all other rows are zero).
    # Host-prepared.
    st.sel = const_pool.tile([97, 256], bf)
    nc.sync.dma_start(st.sel[:], d.sel[:])

    # denominators: local head hl's row at partition 32*hl. Unused
    # partitions are set to 1.0 so the batched ln/exp stays NaN-free.
    st.denom = den_pool.tile([97, S], f32)
    nc.gpsimd.memset(st.denom[:], 1.0)

    st.wqk_t, st.xt_t, st.wv_t = [], [], []
    for k in range(KC):
        w = wqk_pool.tile([128, 512], bf)
        nc.sync.dma_start(w[:], d.wqk[k * 128:(k + 1) * 128, :])
        st.wqk_t.append(w)
        xt = xt_pool.tile([128, S], bf)
        nc.sync.dma_start(xt[:], d.xT[k * 128:(k + 1) * 128, :])
        st.xt_t.append(xt)
    for k in range(KC):
        # wv is first consumed ~30us in; keep it out of the critical
        # DMA prefix that the first qk accumulation chain waits on
        wv = wv_pool.tile([128, 260], bf)
        nc.sync.dma_start(wv[:], d.wv[k * 128:(k + 1) * 128, :])
        st.wv_t.append(wv)
    st.wo_t = []
    for p in range(2):
        w = wo_pool.tile([128, D], bf)
        nc.sync.dma_start(w[:], d.wo[p])
        st.wo_t.append(w)
    if has_bqk:
        st.bqk_sb = []
        for m in range(4):
            t = const_pool.tile([1, 128], bf, name=f"bqk{m}", tag=f"bqk{m}")
            nc.sync.dma_start(t[:], d.bqk[m:m + 1, :])
            st.bqk_sb.append(t)
    if has_bv:
        st.bv_sb = const_pool.tile([1, 260], bf)
        nc.sync.dma_start(st.bv_sb[:], d.bv[:])


def _phase_proj(ctx, tc, nc, has_bqk, has_bv, st):
    """qkv projections.

    qkT[m][r, s]: m-chunks 0..3 = [q pair0 | k pair0 | q pair1 | k pair1];
    within a chunk rows 0-63 = first head of the pair, 64-127 = second.
    v_t[si]: [128, 260] bf16, 4 slots of 65 cols (64 v-cols + ones col).
    """
    from concourse import mybir
    bf = mybir.dt.bfloat16
    f32 = mybir.dt.float32

    qkT_pool = ctx.enter_context(tc.tile_pool(name="qkT", bufs=4))
    v_pool = ctx.enter_context(tc.tile_pool(name="vsb", bufs=NSC))
    st.qkT_t = [qkT_pool.tile([128, S], bf, name="qkT", tag="qkT") for _ in range(4)]
    st.v_t = [v_pool.tile([128, 260], bf, name="vsb", tag="vsb") for _ in range(NSC)]

    def emit_qk(qk_ps, m):
        for s4 in range(NS4):
            ps = qk_ps.tile([128, 512], f32, name="qkps", tag="qkps")
            for k in range(KC):
                nc.tensor.matmul(
                    ps[:],
                    st.wqk_t[k][:, m * 128:(m + 1) * 128],
                    st.xt_t[k][:, s4 * 512:(s4 + 1) * 512],
                    start=(k == 0),
                    stop=(k == KC - 1 and not has_bqk),
                )
            if has_bqk:
                nc.tensor.matmul(
                    ps[:], st.bqk_sb[m][:], st.ones_row[:, :],
                    start=False, stop=True,
                )
            nc.scalar.copy(
                st.qkT_t[m][:, s4 * 512:(s4 + 1) * 512], ps[:])

    with tc.tile_pool(name="qk_ps", bufs=4, space="PSUM") as qk_ps, \
         tc.tile_pool(name="v_ps", bufs=3, space="PSUM") as v_ps:
        for m in range(4):
            emit_qk(qk_ps, m)
        for si in range(NSC):
            ps = v_ps.tile([128, 260], f32)
            for k in range(KC):
                nc.tensor.matmul(
                    ps[:],
                    st.xt_t[k][:, si * 128:(si + 1) * 128],
                    st.wv_t[k][:],
                    start=(k == 0),
                    stop=(k == KC - 1 and not has_bv),
                )
            if has_bv:
                nc.tensor.matmul(
                    ps[:], st.ones_row[0:1, 0:128], st.bv_sb[:],
                    start=False, stop=True,
                )
            nc.vector.tensor_copy(st.v_t[si][:], ps[:])
            for h in range(HPC):
                nc.gpsimd.memset(st.v_t[si][:, 65 * h + 64:65 * h + 65], 1.0)


def _phase_attn(ctx, tc, nc, d, st):
    """Causal attention in 512-query blocks, with normalization and the
    output projection pipelined into the second head-pair's loop.

    Per (pair, qb): for each key chunk kj, both heads' transposed scores go
    into one [128,1024] PSUM tile (h0 cols 0-511, h1 cols 512-1023) via
    tile_position-packed K=64 matmuls -> one exp -> one erb multiply -> two
    PV accumulations, clipped to the causal region at 128-column
    granularity. At block end the unnormalized [64,512] outputs are cast to
    SBUF and the denominator rows collected. Once pair 1 finishes a block,
    both pairs' reciprocals for that block (reciprocal_approx_fast),
    the K=97 selector broadcast matmul, the normalize multiply, and the
    block's four 128-query out-projection chunks are emitted immediately,
    so they overlap the remaining attention blocks instead of forming a
    serial tail. bc/out PSUM tiles share the score pool (8 banks total).
    """
    from concourse import mybir
    bf = mybir.dt.bfloat16
    f32 = mybir.dt.float32
    EXP = mybir.ActivationFunctionType.Exp

    attnU_pool = ctx.enter_context(tc.tile_pool(name="attnU", bufs=2))
    st.attnU = [attnU_pool.tile([128, S], bf, name="attnU", tag="attnU")
                for _ in range(2)]
    attnT_pool = ctx.enter_context(tc.tile_pool(name="attnT", bufs=2))
    st.attnT = [attnT_pool.tile([128, S], bf, name="attnT", tag="attnT")
                for _ in range(2)]
    rec_pool = ctx.enter_context(tc.tile_pool(name="rec", bufs=1))
    recf = rec_pool.tile([97, S], f32)
    rec = rec_pool.tile([97, S], bf)
    osb_pool = ctx.enter_context(tc.tile_pool(name="osb", bufs=4))

    with ExitStack() as cctx:
        sc_ps = cctx.enter_context(tc.tile_pool(name="sc_ps", bufs=2, space="PSUM"))
        pv_ps = cctx.enter_context(tc.tile_pool(name="pv_ps", bufs=4, space="PSUM"))
        erb_pool = cctx.enter_context(tc.tile_pool(name="erb", bufs=8))
        esc_pool = cctx.enter_context(tc.tile_pool(name="esc", bufs=4))
        prob_pool = cctx.enter_context(tc.tile_pool(name="prob", bufs=4))

        for p in range(2):
            qT = st.qkT_t[2 * p]
            kT = st.qkT_t[2 * p + 1]
            for qb in range(4):
                qs = qb * 512
                nkj = 4 * qb + 4
                pv = [pv_ps.tile([65, 512], f32, name="pv", tag="pv")
                      for _ in range(2)]
                for kj in range(nkj):
                    # diagonal key chunks only cover queries >= 128*kj:
                    # clip everything to the causal region at 128-column
                    # granularity (kj==0 has off==0 -> full width).
                    off = max(0, (kj - 4 * qb) * 128)
                    sc = sc_ps.tile([128, 1024], f32, name="sc", tag="sc")
                    for h in range(2):
                        rows = slice(64 * h, 64 * h + 64)
                        nc.tensor.matmul(
                            sc[:, 512 * h + off:512 * h + 512],
                            kT[rows, kj * 128:(kj + 1) * 128],
                            qT[rows, qs + off:qs + 512],
                            start=True, stop=True,
                            tile_position=(64 * h, 0),
                        )
                    esc = esc_pool.tile([128, 1024], bf, name="esc", tag="esc")
                    if off == 0:
                        nc.scalar.activation(esc[:], sc[:], EXP)
                    else:
                        for h in range(2):
                            nc.scalar.activation(
                                esc[:, 512 * h + off:512 * h + 512],
                                sc[:, 512 * h + off:512 * h + 512], EXP)
                    rb = erb_pool.tile([128, 1024], bf, name="erb", tag="erb")
                    for h in range(2):
                        nc.sync.dma_start(
                            rb[:, 512 * h + off:512 * h + 512],
                            d.erb[2 * p + h, kj * 128:(kj + 1) * 128,
                                  qs + off:qs + 512])
                    pr = prob_pool.tile([128, 1024], bf, name="prob", tag="prob")
                    if off == 0:
                        nc.vector.tensor_mul(pr[:], esc[:], rb[:])
                    else:
                        for h in range(2):
                            nc.vector.tensor_mul(
                                pr[:, 512 * h + off:512 * h + 512],
                                esc[:, 512 * h + off:512 * h + 512],
                                rb[:, 512 * h + off:512 * h + 512])
                    for h in range(2):
                        hl = 2 * p + h
                        nc.tensor.matmul(
                            pv[h][:, off:512],
                            st.v_t[kj][:, 65 * hl:65 * hl + 65],
                            pr[:, 512 * h + off:512 * h + 512],
                            start=(kj == 0),
                            stop=(kj == nkj - 1),
                        )
                for h in range(2):
                    hl = 2 * p + h
                    nc.vector.tensor_copy(
                        st.attnU[p][64 * h:64 * h + 64, qs:qs + 512],
                        pv[h][0:64, :])
                    nc.scalar.copy(
                        st.denom[32 * hl:32 * hl + 1, qs:qs + 512],
                        pv[h][64:65, :])

                if p == 1:
                    # normalization + out-projection for this query block
                    nc.vector.reciprocal_approx_fast(
                        recf[:, qs:qs + 512], st.denom[:, qs:qs + 512])
                    nc.vector.tensor_copy(
                        rec[:, qs:qs + 512], recf[:, qs:qs + 512])
                    for pp in range(2):
                        bc = sc_ps.tile([128, 1024], f32, name="sc", tag="sc")
                        nc.tensor.matmul(
                            bc[:, 0:512],
                            st.sel[:, 128 * pp:128 * pp + 128],
                            rec[:, qs:qs + 512],
                            start=True, stop=True,
                        )
                        nc.vector.tensor_mul(
                            st.attnT[pp][:, qs:qs + 512],
                            st.attnU[pp][:, qs:qs + 512],
                            bc[:, 0:512])
                    for si in range(4 * qb, 4 * qb + 4):
                        ot = sc_ps.tile([128, 1024], f32, name="sc", tag="sc")
                        for pp in range(2):
                            for e2 in range(2):
                                nc.tensor.matmul(
                                    ot[:, 512 * e2:512 * e2 + 512],
                                    st.attnT[pp][:, si * 128:(si + 1) * 128],
                                    st.wo_t[pp][:, e2 * 512:(e2 + 1) * 512],
                                    start=(pp == 0), stop=(pp == 1),
                                )
                        osb = osb_pool.tile([128, 1024], bf, name="osb", tag="osb")
                        nc.vector.tensor_copy(osb[:], ot[:])
                        nc.sync.dma_start(
                            d.out[si * 128:(si + 1) * 128, :], osb[:])


_LDW_OPT_INSTALLED = False


def _enable_ldw_opt():
    """walrus ships with --enable-ldw-opt=false; flip it for this process
    (dedupes/hoists LDWEIGHTS). Gated by KERNEL_LDW_OPT=1."""
    global _LDW_OPT_INSTALLED
    if _LDW_OPT_INSTALLED:
        return
    _LDW_OPT_INSTALLED = True
    import os
    if os.environ.get("KERNEL_LDW_OPT", "1") != "1":
        return
    import concourse.bass_utils as bu
    orig = bu.run_command

    def patched(argv, **kwargs):
        argv = ["--enable-ldw-opt=true" if a == "--enable-ldw-opt=false" else a
                for a in argv]
        return orig(argv, **kwargs)

    bu.run_command = patched


def _build_program(has_bqk: bool, has_bv: bool):
    import concourse.tile as tile
    from concourse import bacc, mybir

    bf = mybir.dt.bfloat16
    f32 = mybir.dt.float32

    nc = bacc.Bacc("TRN2", target_bir_lowering=False, debug=False,
                   num_devices=NCORES)

    d = types.SimpleNamespace()
    d.xT = nc.dram_tensor("xT", [D, S], bf, kind="ExternalInput").ap()
    d.wqk = nc.dram_tensor("wqk", [D, 512], bf, kind="ExternalInput").ap()
    d.wv = nc.dram_tensor("wv", [D, 260], bf, kind="ExternalInput").ap()
    d.bqk = nc.dram_tensor("bqk", [4, 128], bf, kind="ExternalInput").ap()
    d.bv = nc.dram_tensor("bv", [1, 260], bf, kind="ExternalInput").ap()
    d.erb = nc.dram_tensor("erb", [HPC, S, S], bf, kind="ExternalInput").ap()
    d.sel = nc.dram_tensor("sel", [97, 256], bf, kind="ExternalInput").ap()
    d.wo = nc.dram_tensor("wo", [2, 128, D], bf, kind="ExternalInput").ap()
    d.out = nc.dram_tensor("out", [S, D], bf, kind="ExternalOutput").ap()

    st = types.SimpleNamespace()
    with tile.TileContext(nc) as tc:
        with ExitStack() as ctx:
            _phase_load(ctx, tc, nc, d, has_bqk, has_bv, st)
            _phase_proj(ctx, tc, nc, has_bqk, has_bv, st)
            _phase_attn(ctx, tc, nc, d, st)

    nc.compile()
    return nc


_PROGRAM_CACHE = {}


def _get_program(has_bqk, has_bv):
    key = (has_bqk, has_bv)
    if key not in _PROGRAM_CACHE:
        _PROGRAM_CACHE[key] = _build_program(has_bqk, has_bv)
    return _PROGRAM_CACHE[key]


_last_results = None  # BassKernelResults of the most recent run (for test.py)


def kernel(x, rel_bias, w_qkv, b_qkv, w_out, b_out, *, trace=False):
    global _last_results
    _install_ntff_hook()
    _enable_ldw_opt()
    from concourse.bass_utils import run_bass_kernel_spmd

    x = np.asarray(x, dtype=np.float32)
    rel_bias = np.asarray(rel_bias, dtype=np.float32)
    w_qkv = np.asarray(w_qkv, dtype=np.float32)
    b_qkv = np.asarray(b_qkv, dtype=np.float32)
    w_out = np.asarray(w_out, dtype=np.float32)
    b_out = np.asarray(b_out, dtype=np.float32)

    wq = w_qkv[:, 0:D]
    wk = w_qkv[:, D:2 * D]
    wv = w_qkv[:, 2 * D:3 * D]
    bq, bk, bv = b_qkv[0:D], b_qkv[D:2 * D], b_qkv[2 * D:3 * D]
    has_bqk = bool(np.any(bq)) or bool(np.any(bk))
    has_bv = bool(np.any(bv))

    nc = _get_program(has_bqk, has_bv)

    sc = 1.0 / math.sqrt(HD)  # folded into the q projection
    xT = [np.ascontiguousarray(x[b].T).astype(_BF16) for b in range(B)]
    tri = np.triu(np.ones((S, S), dtype=np.float32))  # [kj, qi]: qi >= kj

    sel_np = np.zeros((97, 256), dtype=_BF16)
    for hl in range(4):
        p, h = divmod(hl, 2)
        sel_np[32 * hl, 128 * p + 64 * h:128 * p + 64 * h + 64] = 1.0

    in_maps = []
    for c in range(NCORES):
        b, hg = divmod(c, 4)
        hs = [4 * hg + i for i in range(HPC)]

        # wqk columns: [q_h0 | q_h1 | k_h0 | k_h1 | q_h2 | q_h3 | k_h2 | k_h3]
        cols = []
        bqk_rows = []
        for pair in range(2):
            h0, h1 = hs[2 * pair], hs[2 * pair + 1]
            cols += [wq[:, HD * h0:HD * (h0 + 1)] * sc,
                     wq[:, HD * h1:HD * (h1 + 1)] * sc]
            bqk_rows.append(np.concatenate(
                [bq[HD * h0:HD * (h0 + 1)], bq[HD * h1:HD * (h1 + 1)]]) * sc)
            cols += [wk[:, HD * h0:HD * (h0 + 1)],
                     wk[:, HD * h1:HD * (h1 + 1)]]
            bqk_rows.append(np.concatenate(
                [bk[HD * h0:HD * (h0 + 1)], bk[HD * h1:HD * (h1 + 1)]]))
        wqk_c = np.concatenate(cols, axis=1).astype(_BF16)
        bqk_c = np.stack(bqk_rows).astype(_BF16)

        wv_c = np.zeros((D, 260), dtype=np.float32)
        bv_c = np.zeros((1, 260), dtype=np.float32)
        for i, h in enumerate(hs):
            wv_c[:, 65 * i:65 * i + 64] = wv[:, HD * h:HD * (h + 1)]
            bv_c[0, 65 * i:65 * i + 64] = bv[HD * h:HD * (h + 1)]

        erb_c = np.empty((HPC, S, S), dtype=_BF16)
        for i, h in enumerate(hs):
            erb_c[i] = (np.exp(rel_bias[h].T) * tri).astype(_BF16)

        in_maps.append({
            "xT": xT[b],
            "wqk": wqk_c,
            "wv": wv_c.astype(_BF16),
            "bqk": bqk_c,
            "bv": bv_c.astype(_BF16),
            "erb": erb_c,
            "sel": sel_np,
            "wo": np.ascontiguousarray(
                w_out[256 * hg:256 * (hg + 1)].reshape(2, 128, D)).astype(_BF16),
        })

    res = run_bass_kernel_spmd(nc, in_maps, list(range(NCORES)), trace=trace)
    _last_results = res

    out = np.zeros((B, S, D), dtype=np.float32)
    for c in range(NCORES):
        out[c // 4] += res.results[c]["out"].astype(np.float32)
    out += b_out
    return out
